# revision 3
# baseline (speedup 1.0000x reference)
"""Trainium2 Bass kernel for nn_GAT_GCN (gnn_message_passing), 8 NeuronCores.

Optimized v2:
 - Host-precomputed B-folds (as/ad dots, adst1, c2 incl. bg2 fold).
 - Batched leaky-relu+exp per block (one ACT chain on [128, TPB*11]).
 - exv multiplies alternate DVE / Pool engines.
 - T2/x2 table build interleaved into the phase-1 block loop.
 - One combined 896-col exchange row [h2 780 | asrc2 10 | x2h 78 | pad].
 - Phase 3: single merged gather; bgcn/bg2 folded via matmul/c2.
 - Merged x1|x2 pooling rows; one pool-gather family; merged fg1/fg2 head.
"""
import sys
sys.path.insert(0, '/opt/trn_rl_repo')
import numpy as np

N, E, G, F, H = 16384, 131072, 128, 78, 10
NCORE, GPC = 8, 16          # cores, graphs per core
HF = H * F                  # 780
WAUG1 = HF + F + H          # 868 = h1(780) | hgcn(78) | asrc(10)
W2AUG = HF + 2 * H          # 800 = h2 | asrc2 | adst2
CROW = 1024                 # exchange row BYTES: h2 fp8 0:780 | pad | asrc2 fp16
                            # @b784 | x2h fp16 @b804 | pad  (fp16 cols 392:402,
                            # 402:480 via bitcast)
TROW = 896                  # x1f pooling row: x1 780 | x2 78 | pad(zeroed)
XROW = 128                  # x table row, fp16 (256B)
PFW = HF + F                # 858 = pooled feature width (x1|x2)
AS16, XH16 = 392, 402       # fp16-col offsets of asrc2 / x2h in exchange row


def _wrap16(v):
    """dma_gather idx layout: [128, len/16] int16, idx i at (i%16, i//16),
    replicated across the 8 Q7 core groups."""
    v = np.asarray(v, np.int16)
    assert len(v) % 16 == 0
    m = v.reshape(-1, 16).T            # [16, S]
    return np.tile(m, (8, 1)).copy()   # [128, S]


def _f16(a):
    return np.ascontiguousarray(np.asarray(a, np.float32)).astype(np.float16)


def prep(x, edge_index, batch, target, Wg1, as1, ad1, bg1, Wg2, as2, ad2, bg2,
         Wgcn, bgcn, Wfg1, bfg1, Wfg2, bfg2, wconv, bconv, Wxt, bxt,
         W1, b1, W2, b2, Wo, bo):
    x = np.asarray(x, np.float32)
    ei = np.asarray(edge_index, np.int64)
    batch = np.asarray(batch, np.int64)
    target = np.asarray(target, np.float32)
    Wg1 = np.asarray(Wg1, np.float32)
    Wg2 = np.asarray(Wg2, np.float32)

    loops = np.arange(N, dtype=np.int64)
    src = np.concatenate([ei[0], loops])
    dst = np.concatenate([ei[1], loops])

    # graph-aligned core boundaries
    counts = np.bincount(batch, minlength=G)
    node_off = np.concatenate([[0], np.cumsum(counts)])
    n_lo = node_off[np.arange(NCORE) * GPC]
    n_hi = node_off[(np.arange(NCORE) + 1) * GPC]

    # degrees / gcn norm (over full edge list incl self loops)
    deg = np.bincount(dst, minlength=N).astype(np.float64)
    dinv = 1.0 / np.sqrt(deg)
    norm = (dinv[src] * dinv[dst]).astype(np.float32)

    order = np.argsort(dst, kind='stable')
    srcs, dsts, norms = src[order], dst[order], norm[order]

    Lmax = int((n_hi - n_lo).max())
    NBLK = (Lmax + 127) // 128
    NPC = NBLK * 128
    assert NCORE * NPC < 32768

    node_owner = np.searchsorted(n_hi - 1, np.arange(N), side='left')
    node_owner = np.minimum(node_owner, NCORE - 1)
    pad_gid = node_owner * NPC + (np.arange(N) - n_lo[node_owner])

    # per (core, block) edge spans -> uniform TPB
    spans = []
    TPB = 1
    for c in range(NCORE):
        e1 = np.searchsorted(dsts, n_hi[c])
        bl = []
        for b in range(NBLK):
            lo = np.searchsorted(dsts, n_lo[c] + 128 * b)
            hi = np.searchsorted(dsts, min(n_lo[c] + 128 * (b + 1), n_hi[c]))
            if n_lo[c] + 128 * b >= n_hi[c]:
                lo = hi = e1
            bl.append((lo, hi))
            TPB = max(TPB, int((hi - lo + 127) // 128))
        spans.append(bl)
    ET = NBLK * TPB
    ECAP = ET * 128

    PW = int(np.ceil(counts.max() / 16) * 16)   # pool slot width per graph

    # ---------- host-side weight folds ----------
    # B_s1[c, h] = sum_f Wg1[c, h*F+f] * as1[h, f]
    Wg1_3 = Wg1.reshape(F, H, F)
    Bs1 = np.einsum('chf,hf->ch', Wg1_3, np.asarray(as1, np.float32))
    Bd1 = np.einsum('chf,hf->ch', Wg1_3, np.asarray(ad1, np.float32))
    Wg2_3 = Wg2.reshape(HF, H, F)
    Bs2 = np.einsum('chf,hf->ch', Wg2_3, np.asarray(as2, np.float32))
    Bd2 = np.einsum('chf,hf->ch', Wg2_3, np.asarray(ad2, np.float32))

    Wg1cat = np.zeros((128, WAUG1), np.float16)
    Wg1cat[:F, :HF] = _f16(Wg1)
    Wg1cat[:F, HF:HF + F] = _f16(Wgcn)
    Wg1cat[:F, HF + F:WAUG1] = _f16(Bs1)

    W2aug = np.concatenate([Wg2, Bs2, Bd2], axis=1)      # [780, 800]
    W2chunks = np.zeros((7, 128, W2AUG), np.float16)
    for k in range(7):
        r0, r1 = 128 * k, min(128 * (k + 1), HF)
        W2chunks[k, :r1 - r0, :] = _f16(W2aug[r0:r1, :])

    bg1f = np.asarray(bg1, np.float32).reshape(HF)
    c2 = bg1f @ W2aug                                     # [800]
    c2[:HF] += np.asarray(bg2, np.float32).reshape(HF)    # fold bg2 (softmax sums to 1)
    c2row = _f16(c2).reshape(1, W2AUG)

    adst1_full = (x @ Bd1).astype(np.float16)             # [N, 10]

    # merged fg1/fg2 weights over pooled rows [x1max|x2max] + [x1mean|x2mean]
    Wfg1 = np.asarray(Wfg1, np.float32)
    Wfg2 = np.asarray(Wfg2, np.float32)
    wfgm = np.zeros((14, 128, 256), np.float16)
    for j in range(7):
        for p in range(128):
            cidx = 128 * j + p
            if cidx < HF:
                wfgm[j, p, 0:128] = _f16(Wfg1[cidx])
                wfgm[7 + j, p, 0:128] = _f16(Wfg1[HF + cidx])
            elif cidx < PFW:
                wfgm[j, p, 128:256] = _f16(Wfg2[cidx - HF])
                wfgm[7 + j, p, 128:256] = _f16(Wfg2[F + cidx - HF])
    bfg12 = np.concatenate([np.asarray(bfg1, np.float32).reshape(128),
                            np.asarray(bfg2, np.float32).reshape(128)]).reshape(1, 256)

    cores = []
    for c in range(NCORE):
        esrc = np.zeros(ECAP, np.int64)
        s01 = np.zeros((ET, 128, 128), np.float16)
        snrm = np.zeros((ET, 128, 128), np.float16)
        s01t = np.zeros((NBLK, 128, TPB * 128), np.float16)
        for b in range(NBLK):
            lo, hi = spans[c][b]
            ne = hi - lo
            if ne == 0:
                continue
            sl = slice(b * TPB * 128, b * TPB * 128 + ne)
            esrc[sl] = srcs[lo:hi]
            ld = (dsts[lo:hi] - n_lo[c] - 128 * b).astype(np.int64)
            j = np.arange(ne)
            t_loc = j // 128
            e_loc = j % 128
            s01[b * TPB + t_loc, e_loc, ld] = 1.0
            snrm[b * TPB + t_loc, e_loc, ld] = norms[lo:hi].astype(np.float16)
            s01t[b, ld, j] = 1.0
        Lc = int(n_hi[c] - n_lo[c])
        for b in range(NBLK):
            first_pad = max(0, min(128, Lc - 128 * b))
            if first_pad < 128:
                s01[b * TPB, 0, first_pad:] = 1.0

        pool_idx = np.zeros(GPC * PW, np.int64)
        for g in range(GPC):
            gg = c * GPC + g
            a, bnd = node_off[gg] - n_lo[c], node_off[gg + 1] - n_lo[c]
            cnt = bnd - a
            pool_idx[g * PW:g * PW + cnt] = np.arange(a, bnd)
            pool_idx[g * PW + cnt:(g + 1) * PW] = a
        mmean = np.zeros((NBLK, 128, GPC), np.float16)
        for g in range(GPC):
            gg = c * GPC + g
            a, bnd = node_off[gg] - n_lo[c], node_off[gg + 1] - n_lo[c]
            ids = np.arange(a, bnd)
            mmean[ids // 128, ids % 128, g] = np.float16(1.0 / (bnd - a))

        t_win = np.zeros((32, GPC, 608), np.float16)
        tg = target[c * GPC:(c + 1) * GPC, 0, :]
        for k in range(32):
            t_win[k, :, :594] = tg[:, k:k + 594].astype(np.float16)

        adst1c = np.zeros((128, NBLK, H), np.float16)
        for b in range(NBLK):
            nlo = n_lo[c] + 128 * b
            nhi = min(n_lo[c] + 128 * (b + 1), n_hi[c])
            if nhi > nlo:
                adst1c[0:nhi - nlo, b, :] = adst1_full[nlo:nhi]

        s_comb = np.zeros((NBLK, 128, TPB * 256), np.float16)
        for b in range(NBLK):
            for k in range(TPB):
                s_comb[b, :, k * 256:k * 256 + 128] = s01[b * TPB + k]
                s_comb[b, :, k * 256 + 128:k * 256 + 256] = snrm[b * TPB + k]
        cores.append(dict(
            ix_x=_wrap16(esrc),
            ix_t2=_wrap16(pad_gid[esrc]),
            ix_pool=_wrap16(pool_idx),
            s_comb=s_comb, s01t=s01t,
            mmean=mmean, t_win=t_win, adst1=adst1c,
            bconv_rep=np.full((GPC, 1), float(np.asarray(bconv).reshape(-1)[0]),
                              np.float32),
        ))

    x16 = np.zeros((N, XROW), np.float16)
    x16[:, :F] = x.astype(np.float16)

    def pack_rows(Wm, splits, ncol):
        out = np.zeros((len(splits), 128, ncol), np.float16)
        for i, (r0, r1) in enumerate(splits):
            out[i, :r1 - r0, :] = _f16(np.asarray(Wm, np.float32)[r0:r1, :])
        return out

    wxtp = pack_rows(Wxt, [(128 * i, min(128 * (i + 1), 594)) for i in range(5)], 256)
    w1p = pack_rows(W1, [(128 * i, 128 * (i + 1)) for i in range(4)], 512)
    w2p = pack_rows(W2, [(128 * i, 128 * (i + 1)) for i in range(4)], 256)
    wop = pack_rows(Wo, [(0, 128), (128, 256)], 1)

    wgcn_s = np.zeros((128, F), np.float16)
    wgcn_s[:F] = _f16(Wgcn)
    bgcn_col = np.zeros((128, 1), np.float32)
    bgcn_col[:F, 0] = np.asarray(bgcn, np.float32)

    shared = dict(
        x16=x16, Wg1cat=Wg1cat, W2chunks=W2chunks, c2row=c2row,
        wgcn_s=wgcn_s, bgcn_col=bgcn_col,
        bgcn16=_f16(np.asarray(bgcn, np.float32)).reshape(1, F),
        wfgm=wfgm, bfg12=bfg12,
        wxtp=wxtp, bxt=np.asarray(bxt, np.float32).reshape(1, 256),
        w1p=w1p, b1=np.asarray(b1, np.float32).reshape(1, 512),
        w2p=w2p, b2=np.asarray(b2, np.float32).reshape(1, 256),
        wop=wop, bo_rep=np.full((GPC, 1), float(np.asarray(bo).reshape(-1)[0]),
                                np.float32),
        w_sel=np.zeros((32, GPC, GPC), np.float16),
    )
    wcol = _f16(np.asarray(wconv).reshape(-1))
    for g in range(GPC):
        shared['w_sel'][:, g, g] = wcol

    meta = dict(NBLK=NBLK, NPC=NPC, TPB=int(TPB), ET=ET, ECAP=ECAP, PW=PW,
                n_lo=n_lo, n_hi=n_hi)
    return meta, shared, cores


import concourse.bass as bass
import concourse.bacc as bacc
import concourse.mybir as mybir
from concourse import library_config
from concourse.tile import TileContext
from concourse.masks import make_identity
from concourse.bass_utils import run_bass_kernel_spmd

F16 = mybir.dt.float16
F32 = mybir.dt.float32
F8 = mybir.dt.float8e4
U8 = mybir.dt.uint8
I16 = mybir.dt.int16
AX = mybir.AxisListType.X
ALU = mybir.AluOpType
AF = mybir.ActivationFunctionType


def build(meta):
    NBLK, NPC, TPB, ET, ECAP, PW = (int(meta[k]) for k in
                                    ['NBLK', 'NPC', 'TPB', 'ET', 'ECAP', 'PW'])
    EPB = TPB * 128                       # edges per block
    TL = TPB * 11                         # batched logit width (phase 1)
    TL3 = TPB * 10                        # batched logit width (phase 3)
    nc = bacc.Bacc()

    dp = lambda n, s, d: nc.declare_dram_parameter(n, list(s), d, isOutput=False)
    # per-core inputs
    x16 = dp('x16', [N, XROW], F16)
    ix_x = dp('ix_x', [128, ECAP // 16], I16)
    ix_t2 = dp('ix_t2', [128, ECAP // 16], I16)
    ix_pool = dp('ix_pool', [128, GPC * PW // 16], I16)
    scomb_d = dp('s_comb', [NBLK, 128, TPB * 256], F16)
    s01t_d = dp('s01t', [NBLK, 128, EPB], F16)
    mmean_d = dp('mmean', [NBLK, 128, GPC], F16)
    twin_d = dp('t_win', [32, GPC, 608], F16)
    bconv_rep = dp('bconv_rep', [GPC, 1], F32)
    adst1_d = dp('adst1', [128, NBLK, H], F16)
    # shared weights
    wg1cat = dp('Wg1cat', [128, WAUG1], F16)
    w2ch = dp('W2chunks', [7, 128, W2AUG], F16)
    c2_d = dp('c2row', [1, W2AUG], F16)
    wgcn = dp('wgcn_s', [128, F], F16)
    bgcn_col = dp('bgcn_col', [128, 1], F32)
    bgcn16_d = dp('bgcn16', [1, F], F16)
    wfgm_d = dp('wfgm', [14, 128, 256], F16)
    bfg12_d = dp('bfg12', [1, 256], F32)
    wxtp = dp('wxtp', [5, 128, 256], F16)
    bxt = dp('bxt', [1, 256], F32)
    w1p = dp('w1p', [4, 128, 512], F16)
    b1 = dp('b1', [1, 512], F32)
    w2p = dp('w2p', [4, 128, 256], F16)
    b2 = dp('b2', [1, 256], F32)
    wop = dp('wop', [2, 128, 1], F16)
    bo_rep = dp('bo_rep', [GPC, 1], F32)
    wsel_d = dp('w_sel', [32, GPC, GPC], F16)

    out_d = nc.declare_dram_parameter('out', [GPC, 1], F32, isOutput=True)

    # internal DRAM (uint8: rows mix fp8 h2 with fp16 asrc2/x2h fields)
    comb_shard = nc.dram_tensor('comb_shard', [NPC, CROW], U8)
    comb_full = nc.dram_tensor('comb_full', [8 * NPC, CROW], U8,
                               addr_space="Shared")
    x1f_dram = nc.dram_tensor('x1f_dram', [NPC, TROW], F16)

    RG = [list(range(8))]

    with TileContext(nc) as tc:
        nc.gpsimd.load_library(library_config.mlp)

        with tc.tile_pool(name="persist", bufs=1) as pp:
            # ---- early-critical persistent loads
            w1aug_s = pp.tile([128, WAUG1], F16, tag="w1aug")
            nc.sync.dma_start(out=w1aug_s[:], in_=wg1cat[:])
            adst1_s = pp.tile([128, NBLK, H], F16, tag="adst1")
            nc.sync.dma_start(out=adst1_s[:], in_=adst1_d[:])
            ixx_s = pp.tile([128, ECAP // 16], I16, tag="ixx")
            nc.sync.dma_start(out=ixx_s[:], in_=ix_x[:])
            w2aug_s = pp.tile([128, 7, W2AUG], F16, tag="w2aug")
            for k in range(7):
                nc.sync.dma_start(out=w2aug_s[:, k, :], in_=w2ch[k])
            c2_s = pp.tile([1, W2AUG], F16, tag="c2")
            nc.sync.dma_start(out=c2_s[:], in_=c2_d[:])
            wgcn_s = pp.tile([128, F], F16, tag="wgcn")
            nc.sync.dma_start(out=wgcn_s[:], in_=wgcn[:])
            bgcnc_s = pp.tile([128, 1], F32, tag="bgcnc")
            nc.sync.dma_start(out=bgcnc_s[:], in_=bgcn_col[:])

            ident_s = pp.tile([128, 128], F16, tag="ident")
            make_identity(nc, ident_s[:])
            ones_s = pp.tile([1, 128], F16, tag="ones")
            nc.vector.memset(ones_s[:], 1.0)

            adst2_s = pp.tile([128, NBLK, H], F16, tag="adst2")
            x1f_s = pp.tile([128, NBLK, TROW], F16, tag="x1f")
            nc.gpsimd.memset(x1f_s[:, :, PFW:TROW], 0.0)

            # ---------------- phase 1: GAT1+GCN1 edge loop + T2/x2 build
            # software-pipelined: block b's tail (T2/x2 build) is emitted while
            # block b+1's act chain runs; projection runs one tile ahead.
            LGO = WAUG1  # logit slots live at psum cols [868:868+TL] (bank 1)
            with tc.tile_pool(name="p1g", bufs=3) as p1g, \
                 tc.tile_pool(name="p1d", bufs=2) as p1d, \
                 tc.tile_pool(name="p1", bufs=3) as p1, \
                 tc.tile_pool(name="p1b", bufs=2) as p1b, \
                 tc.tile_pool(name="p1sa", bufs=3, space="PSUM") as p1sa, \
                 tc.tile_pool(name="p1sb", bufs=3, space="PSUM") as p1sb, \
                 tc.tile_pool(name="p1acc", bufs=1, space="PSUM") as p1acc:
                blk = {}

                def p1_gather(b):
                    xgt = p1g.tile([128, 1, EPB], F16, tag="xgt")
                    nc.gpsimd.dma_gather(
                        out_ap=xgt[:], in_ap=x16[:],
                        idxs_ap=ixx_s[:, b * (EPB // 16):(b + 1) * (EPB // 16)],
                        num_idxs=EPB, num_idxs_reg=EPB, elem_size=XROW,
                        transpose=True, single_packet=False)
                    return xgt

                def p1_head(b, xgt):
                    st = {'xgt': xgt}
                    scomb_b = p1d.tile([128, TPB * 256], F16, tag="scombb")
                    nc.sync.dma_start(out=scomb_b[:], in_=scomb_d[b])
                    s01t_b = p1d.tile([128, EPB], F16, tag="s01tb")
                    nc.sync.dma_start(out=s01t_b[:], in_=s01t_d[b])
                    ps_out = p1acc.tile([128, 1024], F32, space="PSUM",
                                        tag="psout", name="psout")[:]
                    ps_logit = ps_out[:, LGO:LGO + TL].rearrange(
                        "p (t e) -> p t e", t=TPB)
                    nc.vector.memset(ps_logit[:, :, 10:11], 0.0)
                    for k in range(TPB):
                        nc.tensor.matmul(out=ps_logit[:, k, 0:10],
                                         lhsT=xgt[:, 0, 128 * k:128 * (k + 1)],
                                         rhs=w1aug_s[:, HF + F:WAUG1],
                                         start=True, stop=False)
                        nc.tensor.matmul(out=ps_logit[:, k, 0:10],
                                         lhsT=s01t_b[:, 128 * k:128 * (k + 1)],
                                         rhs=adst1_s[:, b, :],
                                         start=False, stop=True)
                    st.update(scomb=scomb_b, s01t=s01t_b, ps_out=ps_out)
                    return st

                def p1_act(st):
                    ps_out = st['ps_out']
                    lr02 = p1.tile([128, TL], F32, tag="lr02")
                    nc.scalar.activation(out=lr02[:], in_=ps_out[:, LGO:LGO + TL],
                                         func=AF.Copy, scale=0.2)
                    lr = p1.tile([128, TL], F32, tag="lr")
                    nc.vector.tensor_tensor(out=lr[:], in0=ps_out[:, LGO:LGO + TL],
                                            in1=lr02[:], op=ALU.max)
                    ex_blk = p1.tile([128, TPB, 11], F16, tag="exb")
                    nc.scalar.activation(
                        out=ex_blk[:].rearrange("p t e -> p (t e)"),
                        in_=lr[:], func=AF.Exp)
                    st['ex'] = ex_blk

                HS = 6 * F  # 468, head-aligned psum split point

                def p1_proj(st, k):
                    lhs = st['xgt'][:, 0, 128 * k:128 * (k + 1)]
                    ps1a = p1sa.tile([128, HS], F32, space="PSUM", tag="ps1a")
                    nc.tensor.matmul(out=ps1a[:], lhsT=lhs,
                                     rhs=w1aug_s[:, 0:HS], start=True, stop=True)
                    ps1b = p1sb.tile([128, PFW - HS], F32, space="PSUM",
                                     tag="ps1b")
                    nc.tensor.matmul(out=ps1b[:], lhsT=lhs,
                                     rhs=w1aug_s[:, HS:PFW], start=True, stop=True)
                    return ps1a, ps1b

                def p1_sub2(st, b):
                    ps_out, ex_blk, scomb_b = st['ps_out'], st['ex'], st['scomb']
                    pq = [p1_proj(st, 0), p1_proj(st, 1)]
                    for k in range(TPB):
                        ps1a, ps1b = pq[k % 2]
                        if k + 2 < TPB:
                            pq[k % 2] = p1_proj(st, k + 2)
                        exv = p1.tile([128, PFW], F16, tag="exv")
                        if k % 2 == 0:
                            # DVE multiplies straight out of PSUM
                            nc.vector.tensor_tensor(
                                out=exv[:, 0:HS].rearrange("p (h f) -> p h f", h=6),
                                in0=ps1a[:].rearrange("p (h f) -> p h f", h=6),
                                in1=ex_blk[:, k, 0:6, None].to_broadcast([128, 6, F]),
                                op=ALU.mult)
                            nc.vector.tensor_tensor(
                                out=exv[:, HS:PFW].rearrange("p (h f) -> p h f", h=5),
                                in0=ps1b[:].rearrange("p (h f) -> p h f", h=5),
                                in1=ex_blk[:, k, 6:11, None].to_broadcast([128, 5, F]),
                                op=ALU.mult)
                        else:
                            # Pool cannot read PSUM on HW: ACT stages to SBUF
                            h1s = p1.tile([128, PFW], F16, tag="h1s")
                            nc.scalar.activation(out=h1s[:, 0:HS], in_=ps1a[:],
                                                 func=AF.Copy)
                            nc.scalar.activation(out=h1s[:, HS:PFW], in_=ps1b[:],
                                                 func=AF.Copy)
                            nc.gpsimd.tensor_tensor(
                                out=exv[:].rearrange("p (h f) -> p h f", h=H + 1),
                                in0=h1s[:].rearrange("p (h f) -> p h f", h=H + 1),
                                in1=ex_blk[:, k, :, None].to_broadcast(
                                    [128, H + 1, F]),
                                op=ALU.mult)
                        s01_t = scomb_b[:, k * 256:k * 256 + 128]
                        snrm_t = scomb_b[:, k * 256 + 128:k * 256 + 256]
                        st_, sp = (k == 0), (k == TPB - 1)
                        # bank0 [0:512]: own group; bank1 [512:868]: ONE group
                        nc.tensor.matmul(out=ps_out[:, 0:512], lhsT=s01_t,
                                         rhs=exv[:, 0:512], start=st_, stop=sp)
                        nc.tensor.matmul(out=ps_out[:, 512:HF], lhsT=s01_t,
                                         rhs=exv[:, 512:HF], start=st_, stop=False)
                        nc.tensor.matmul(out=ps_out[:, HF:HF + H], lhsT=s01_t,
                                         rhs=ex_blk[:, k, 0:10], start=False,
                                         stop=False)
                        nc.tensor.matmul(out=ps_out[:, HF + H:WAUG1], lhsT=snrm_t,
                                         rhs=exv[:, HF:PFW], start=False, stop=sp)

                def p1_tail(st, b):
                    ps_out = st['ps_out']
                    rec = p1.tile([128, H], F32, tag="rec")
                    nc.vector.reciprocal(out=rec[:], in_=ps_out[:, HF:HF + H])
                    x1loc = p1b.tile([128, HF], F16, tag="x1loc")
                    nc.vector.tensor_tensor(
                        out=x1loc[:].rearrange("p (h f) -> p h f", h=H),
                        in0=ps_out[:, 0:HF].rearrange("p (h f) -> p h f", h=H),
                        in1=rec[:, :, None].to_broadcast([128, H, F]),
                        op=ALU.mult)
                    agg1b = p1b.tile([128, F], F16, tag="agg1b")
                    nc.scalar.activation(out=agg1b[:], in_=ps_out[:, HF + H:WAUG1],
                                         func=AF.Copy)

                    stage_u = p1b.tile([128, CROW], U8, tag="stage")
                    stage = stage_u[:].bitcast(F8)
                    stage16 = stage_u[:].bitcast(F16)
                    nc.gpsimd.memset(stage16[:, XH16 + F:CROW // 2], 0.0)
                    nc.gpsimd.memset(stage16[:, HF // 2:AS16], 0.0)
                    # one psum slot holds all 8 transposes via f16 sub-slots
                    psTa_t = p1sa.tile([128, HS], F32, space="PSUM", tag="ps1a",
                                       name="psTa_t")
                    psTa = psTa_t[:].bitcast(F16)
                    nc.tensor.transpose(out=psTa[:F, 512:640], in_=agg1b[:],
                                        identity=ident_s[:])
                    x2lt = p1b.tile([128, 128], F16, tag="x2lt")
                    nc.vector.tensor_scalar(out=x2lt[:F, :], in0=psTa[:F, 512:640],
                                            scalar1=bgcnc_s[:F, :], scalar2=None,
                                            op0=ALU.add)
                    x1tb = p1b.tile([128, 7, 128], F16, tag="x1tb")
                    nc.gpsimd.memset(x1tb[:, 6, :], 0.0)
                    for j in range(7):
                        c0, c1 = 128 * j, min(128 * (j + 1), HF)
                        sl = (j % 4) * 128
                        nc.tensor.transpose(out=psTa[:c1 - c0, sl:sl + 128],
                                            in_=x1loc[:, c0:c1],
                                            identity=ident_s[:])
                        nc.vector.tensor_copy(out=x1tb[0:c1 - c0, j, :],
                                              in_=psTa[:c1 - c0, sl:sl + 128])
                    # x2h = (agg1 + bgcn) @ Wgcn, in a ps1b slot
                    ps_x2 = p1sb.tile([128, PFW - HS], F32, space="PSUM",
                                      tag="ps1b", name="ps_x2")
                    nc.tensor.matmul(out=ps_x2[:, 0:F], lhsT=x2lt[:F, :],
                                     rhs=wgcn_s[:F, :], start=True, stop=True)
                    nc.scalar.activation(out=stage16[:, XH16:XH16 + F],
                                         in_=ps_x2[:, 0:F], func=AF.Copy)
                    # T2 = x1 @ W2aug + c2 (c2 includes bg1-fold + bg2)
                    ps_t2a = p1sa.tile([128, HS], F32, space="PSUM", tag="ps1a",
                                       name="ps_t2a")
                    ps_t2b = p1sb.tile([128, PFW - HS], F32, space="PSUM",
                                       tag="ps1b", name="ps_t2b")
                    for j in range(7):
                        nc.tensor.matmul(out=ps_t2a[:], lhsT=x1tb[:, j, :],
                                         rhs=w2aug_s[:, j, 0:HS],
                                         start=(j == 0), stop=False)
                        nc.tensor.matmul(out=ps_t2b[:, 0:W2AUG - HS],
                                         lhsT=x1tb[:, j, :],
                                         rhs=w2aug_s[:, j, HS:W2AUG],
                                         start=(j == 0), stop=False)
                    nc.tensor.matmul(out=ps_t2a[:], lhsT=ones_s[:],
                                     rhs=c2_s[:, 0:HS], start=False, stop=True)
                    nc.tensor.matmul(out=ps_t2b[:, 0:W2AUG - HS], lhsT=ones_s[:],
                                     rhs=c2_s[:, HS:W2AUG], start=False, stop=True)
                    # h2 -> fp8; asrc2 -> fp16 slot; adst2 -> sbuf table
                    nc.scalar.activation(out=stage[:, 0:HS],
                                         in_=ps_t2a[:], func=AF.Copy)
                    nc.scalar.activation(out=stage[:, HS:HF],
                                         in_=ps_t2b[:, 0:HF - HS], func=AF.Copy)
                    nc.scalar.activation(out=stage16[:, AS16:AS16 + H],
                                         in_=ps_t2b[:, HF - HS:HF + H - HS],
                                         func=AF.Copy)
                    nc.scalar.activation(out=adst2_s[:, b, :],
                                         in_=ps_t2b[:, HF + H - HS:W2AUG - HS],
                                         func=AF.Copy)
                    nc.sync.dma_start(out=comb_shard[128 * b:128 * (b + 1), :],
                                      in_=stage_u[:])

                xg_next = p1_gather(0)
                for b in range(NBLK):
                    xg = xg_next
                    cur = p1_head(b, xg)
                    if b + 1 < NBLK:
                        xg_next = p1_gather(b + 1)
                    p1_act(cur)
                    if b > 0:
                        p1_tail(blk[b - 1], b - 1)
                    p1_sub2(cur, b)
                    blk[b] = cur
                p1_tail(blk[NBLK - 1], NBLK - 1)

            # ---------------- exchange + conv branch + late loads
            with tc.tile_pool(name="p2", bufs=2) as p2, \
                 tc.tile_pool(name="p2s", bufs=1, space="PSUM") as p2s, \
                 tc.tile_pool(name="p2t", bufs=2, space="PSUM") as p2t:
                nc.gpsimd.collective_compute(
                    "AllGather", ALU.bypass, replica_groups=RG,
                    ins=[comb_shard[:]], outs=[comb_full[:]])

                # late persistent loads (fill the collective window)
                ixt2_s = pp.tile([128, ECAP // 16], I16, tag="ixt2")
                nc.sync.dma_start(out=ixt2_s[:], in_=ix_t2[:])
                ixp_s = pp.tile([128, GPC * PW // 16], I16, tag="ixp")
                nc.sync.dma_start(out=ixp_s[:], in_=ix_pool[:])
                bgcn16_s = pp.tile([1, F], F16, tag="bgcn16")
                nc.sync.dma_start(out=bgcn16_s[:], in_=bgcn16_d[:])
                mmean_s = pp.tile([128, NBLK, GPC], F16, tag="mmean")
                for b in range(NBLK):
                    nc.sync.dma_start(out=mmean_s[:, b, :], in_=mmean_d[b])
                wsel_s = pp.tile([32, GPC, GPC], F16, tag="wsel")
                nc.sync.dma_start(out=wsel_s[:], in_=wsel_d[:])
                bconv_s = pp.tile([GPC, 1], F32, tag="bconv")
                nc.sync.dma_start(out=bconv_s[:], in_=bconv_rep[:])
                wfg_s = pp.tile([128, 14, 256], F16, tag="wfg")
                for i in range(14):
                    nc.sync.dma_start(out=wfg_s[:, i, :], in_=wfgm_d[i])
                wxt_s = pp.tile([128, 5, 256], F16, tag="wxt")
                for i in range(5):
                    nc.sync.dma_start(out=wxt_s[:, i, :], in_=wxtp[i])
                w1_s = pp.tile([128, 4, 512], F16, tag="w1")
                for i in range(4):
                    nc.sync.dma_start(out=w1_s[:, i, :], in_=w1p[i])
                w2_s = pp.tile([128, 4, 256], F16, tag="w2")
                for i in range(4):
                    nc.sync.dma_start(out=w2_s[:, i, :], in_=w2p[i])
                wo_s = pp.tile([128, 2, 1], F16, tag="wo")
                for i in range(2):
                    nc.sync.dma_start(out=wo_s[:, i, :], in_=wop[i])
                bias_s = {}
                for nm, t, w in [('bfg12', bfg12_d, 256), ('bxt', bxt, 256),
                                 ('b1', b1, 512), ('b2', b2, 256)]:
                    bias_s[nm] = pp.tile([GPC, w], F32, tag="bias_" + nm,
                                         name="bias_" + nm)
                    nc.sync.dma_start(out=bias_s[nm][:],
                                      in_=t[:].to_broadcast([GPC, w]))
                bo_s = pp.tile([GPC, 1], F32, tag="bo")
                nc.sync.dma_start(out=bo_s[:], in_=bo_rep[:])

                # conv branch (independent of the exchange)
                twin_s = p2.tile([32, GPC, 608], F16, tag="twin", bufs=1)
                nc.sync.dma_start(out=twin_s[:], in_=twin_d[:])
                ps_ya = p2s.tile([GPC, 512], F32, space="PSUM", tag="psya")
                ps_yb = p2s.tile([GPC, 96], F32, space="PSUM", tag="psyb")
                for g in range(GPC):
                    nc.tensor.matmul(out=ps_ya[:], lhsT=wsel_s[:, g, :],
                                     rhs=twin_s[:, g, 0:512], start=(g == 0),
                                     stop=(g == GPC - 1))
                    nc.tensor.matmul(out=ps_yb[:], lhsT=wsel_s[:, g, :],
                                     rhs=twin_s[:, g, 512:608], start=(g == 0),
                                     stop=(g == GPC - 1))
                y_s = p2.tile([GPC, 608], F16, tag="ys")
                nc.vector.tensor_scalar(out=y_s[:, 0:512], in0=ps_ya[:],
                                        scalar1=bconv_s[:], scalar2=0.0,
                                        op0=ALU.add, op1=ALU.max)
                nc.vector.tensor_scalar(out=y_s[:, 512:608], in0=ps_yb[:],
                                        scalar1=bconv_s[:], scalar2=0.0,
                                        op0=ALU.add, op1=ALU.max)
                yt_s = p2.tile([128, 5, GPC], F16, tag="yt", bufs=1)
                nc.gpsimd.memset(yt_s[:], 0.0)
                for i in range(5):
                    c0, c1 = 128 * i, min(128 * (i + 1), 608)
                    psT = p2t.tile([128, 128], F16, space="PSUM", tag="psT")
                    nc.tensor.transpose(out=psT[:c1 - c0, :GPC], in_=y_s[:, c0:c1],
                                        identity=ident_s[:GPC, :GPC])
                    nc.scalar.activation(out=yt_s[0:c1 - c0, i, :],
                                         in_=psT[:c1 - c0, :GPC], func=AF.Copy)
                ps_xt = p2s.tile([GPC, 256], F32, space="PSUM", tag="psxt")
                for i in range(5):
                    nc.tensor.matmul(out=ps_xt[:], lhsT=yt_s[:, i, :],
                                     rhs=wxt_s[:, i, :], start=(i == 0),
                                     stop=(i == 4))
                xt_s = p2.tile([GPC, 256], F16, tag="xts")
                nc.vector.tensor_tensor(out=xt_s[:], in0=ps_xt[:],
                                        in1=bias_s['bxt'][:], op=ALU.add)
                xtT_s = pp.tile([128, 2, GPC], F16, tag="xtT")
                for i in range(2):
                    psT = p2t.tile([128, 128], F16, space="PSUM", tag="psT")
                    nc.tensor.transpose(out=psT[:, :GPC],
                                        in_=xt_s[:, 128 * i:128 * (i + 1)],
                                        identity=ident_s[:GPC, :GPC])
                    nc.scalar.activation(out=xtT_s[:, i, :], in_=psT[:, :GPC],
                                         func=AF.Copy)

            # ---------------- phase 3: GAT2 + GCN2 edge loop (pipelined)
            with tc.tile_pool(name="p3g", bufs=4) as p3g, \
                 tc.tile_pool(name="p3d", bufs=2) as p3d, \
                 tc.tile_pool(name="p3", bufs=3) as p3, \
                 tc.tile_pool(name="p3acc", bufs=2, space="PSUM") as p3acc:
                blk3 = {}

                def p3_gather(b):
                    # F16-typed gather of the u8 rows (1024B): avoids the
                    # sub-512B-dtype 2x DMA penalty the u8 path pays.
                    v2g_u = p3g.tile([128, TPB, CROW // 2], F16, tag="v2g")
                    nc.gpsimd.dma_gather(
                        out_ap=v2g_u[:], in_ap=comb_full[:].bitcast(F16),
                        idxs_ap=ixt2_s[:, b * (EPB // 16):(b + 1) * (EPB // 16)],
                        num_idxs=EPB, num_idxs_reg=EPB, elem_size=CROW // 2,
                        single_packet=False)
                    return v2g_u

                def p3_head(b, v2g_u):
                    st = {'v2g_u': v2g_u}
                    scomb_b = p3d.tile([128, TPB * 256], F16, tag="scombb3")
                    nc.sync.dma_start(out=scomb_b[:], in_=scomb_d[b])
                    s01t_b = p3d.tile([128, EPB], F16, tag="s01tb3")
                    nc.sync.dma_start(out=s01t_b[:], in_=s01t_d[b])
                    ps_out = p3acc.tile([128, 1024], F32, space="PSUM",
                                        tag="psout3", name="psout3")[:]
                    ps_logit = ps_out[:, LGO:LGO + TL3].rearrange(
                        "p (t e) -> p t e", t=TPB)
                    for k in range(TPB):
                        nc.tensor.matmul(out=ps_logit[:, k, :],
                                         lhsT=s01t_b[:, 128 * k:128 * (k + 1)],
                                         rhs=adst2_s[:, b, :], start=True, stop=False)
                        nc.tensor.matmul(out=ps_logit[:, k, :], lhsT=ident_s[:],
                                         rhs=v2g_u[:, k, AS16:AS16 + H],
                                         start=False, stop=True)
                    st.update(scomb=scomb_b, ps_out=ps_out)
                    return st

                def p3_act(st):
                    ps_out = st['ps_out']
                    lr02 = p3.tile([128, TL3], F32, tag="lr023")
                    nc.scalar.activation(out=lr02[:], in_=ps_out[:, LGO:LGO + TL3],
                                         func=AF.Copy, scale=0.2)
                    lr = p3.tile([128, TL3], F32, tag="lr3")
                    nc.vector.tensor_tensor(out=lr[:], in0=ps_out[:, LGO:LGO + TL3],
                                            in1=lr02[:], op=ALU.max)
                    ex_blk = p3.tile([128, TPB, 10], F16, tag="exb3")
                    nc.scalar.activation(
                        out=ex_blk[:].rearrange("p t e -> p (t e)"),
                        in_=lr[:], func=AF.Exp)
                    st['ex'] = ex_blk

                def p3_sub2(st, b):
                    ps_out, ex_blk, scomb_b, v2g_u = (st['ps_out'], st['ex'],
                                                      st['scomb'], st['v2g_u'])
                    for k in range(TPB):
                        exv = p3.tile([128, HF], F16, tag="exv3")
                        eng = nc.vector if k % 2 == 0 else nc.gpsimd
                        eng.tensor_tensor(
                            out=exv[:].rearrange("p (h f) -> p h f", h=H),
                            in0=v2g_u[:, k, 0:HF // 2].bitcast(F8)
                                .rearrange("p (h f) -> p h f", h=H),
                            in1=ex_blk[:, k, :, None].to_broadcast([128, H, F]),
                            op=ALU.mult)
                        s01_t = scomb_b[:, k * 256:k * 256 + 128]
                        snrm_t = scomb_b[:, k * 256 + 128:k * 256 + 256]
                        st_, sp = (k == 0), (k == TPB - 1)
                        nc.tensor.matmul(out=ps_out[:, 0:512], lhsT=s01_t,
                                         rhs=exv[:, 0:512], start=st_, stop=sp)
                        nc.tensor.matmul(out=ps_out[:, 512:HF], lhsT=s01_t,
                                         rhs=exv[:, 512:HF], start=st_, stop=False)
                        nc.tensor.matmul(out=ps_out[:, HF:HF + H], lhsT=s01_t,
                                         rhs=ex_blk[:, k, :], start=False,
                                         stop=False)
                        nc.tensor.matmul(out=ps_out[:, HF + H:WAUG1], lhsT=snrm_t,
                                         rhs=v2g_u[:, k, XH16:XH16 + F],
                                         start=False, stop=False)
                    # fold bgcn into the GCN2 accumulation, then stop bank-1 group
                    nc.tensor.matmul(out=ps_out[:, HF + H:WAUG1], lhsT=ones_s[:],
                                     rhs=bgcn16_s[:], start=False, stop=True)

                def p3_tail(st, b):
                    ps_out = st['ps_out']
                    rec = p3.tile([128, H], F32, tag="rec3")
                    nc.vector.reciprocal(out=rec[:], in_=ps_out[:, HF:HF + H])
                    u_s = p3.tile([128, HF], F16, tag="us")
                    nc.vector.tensor_tensor(
                        out=u_s[:].rearrange("p (h f) -> p h f", h=H),
                        in0=ps_out[:, 0:HF].rearrange("p (h f) -> p h f", h=H),
                        in1=rec[:, :, None].to_broadcast([128, H, F]),
                        op=ALU.mult)
                    # bg2 already folded via c2; just relu into pooling rows
                    nc.scalar.activation(out=x1f_s[:, b, 0:HF], in_=u_s[:],
                                         func=AF.Relu)
                    nc.scalar.activation(out=x1f_s[:, b, HF:PFW],
                                         in_=ps_out[:, HF + H:WAUG1], func=AF.Relu)
                    nc.sync.dma_start(out=x1f_dram[128 * b:128 * (b + 1), :],
                                      in_=x1f_s[:, b, :])

                gq = {0: p3_gather(0), 1: p3_gather(1)}
                for b in range(NBLK):
                    cur = p3_head(b, gq.pop(b))
                    if b + 2 < NBLK:
                        gq[b + 2] = p3_gather(b + 2)
                    p3_act(cur)
                    if b > 0:
                        p3_tail(blk3[b - 1], b - 1)
                    p3_sub2(cur, b)
                    blk3[b] = cur
                p3_tail(blk3[NBLK - 1], NBLK - 1)

            # ---------------- phase 4: pooling + head
            with tc.tile_pool(name="p4", bufs=2) as p4, \
                 tc.tile_pool(name="p4s", bufs=1, space="PSUM") as p4s, \
                 tc.tile_pool(name="p4t", bufs=2, space="PSUM") as p4t:
                gmaxT = pp.tile([128, 7, GPC], F16, tag="gmaxT")
                CH = GPC // 2
                for h in range(2):
                    slab = p4.tile([128, 7, CH * PW], F16, tag="slab")
                    nc.gpsimd.dma_gather(
                        out_ap=slab[:], in_ap=x1f_dram[:],
                        idxs_ap=ixp_s[:, h * (CH * PW // 16):(h + 1) * (CH * PW // 16)],
                        num_idxs=CH * PW, num_idxs_reg=CH * PW, elem_size=TROW,
                        transpose=True, single_packet=False)
                    for g in range(0, CH, 2):
                        nc.vector.tensor_reduce(
                            out=gmaxT[:, :, h * CH + g:h * CH + g + 2],
                            in_=slab[:, :, g * PW:(g + 2) * PW].rearrange(
                                "p j (g w) -> p j g w", g=2),
                            op=ALU.max, axis=AX)
                # means via matmul against SBUF x1f, then transpose
                ps_m = p4s.tile([GPC, PFW], F32, space="PSUM", tag="psm")
                for b in range(NBLK):
                    nc.tensor.matmul(out=ps_m[:, 0:512], lhsT=mmean_s[:, b, :],
                                     rhs=x1f_s[:, b, 0:512], start=(b == 0),
                                     stop=(b == NBLK - 1))
                    nc.tensor.matmul(out=ps_m[:, 512:PFW], lhsT=mmean_s[:, b, :],
                                     rhs=x1f_s[:, b, 512:PFW], start=(b == 0),
                                     stop=(b == NBLK - 1))
                mean_s = p4.tile([GPC, PFW], F16, tag="means")
                nc.scalar.activation(out=mean_s[:], in_=ps_m[:], func=AF.Copy)
                gmeanT = pp.tile([128, 7, GPC], F16, tag="gmeanT")
                nc.gpsimd.memset(gmeanT[:], 0.0)
                for i in range(7):
                    c0, c1 = 128 * i, min(128 * (i + 1), PFW)
                    psT = p4t.tile([128, 128], F16, space="PSUM", tag="psT4h")
                    nc.tensor.transpose(out=psT[:c1 - c0, :GPC],
                                        in_=mean_s[:, c0:c1],
                                        identity=ident_s[:GPC, :GPC])
                    nc.scalar.activation(out=gmeanT[0:c1 - c0, i, :],
                                         in_=psT[:c1 - c0, :GPC], func=AF.Copy)

                def head_mm(ps, chunks, rhs_tile, nw):
                    n = len(chunks)
                    for i, ch in enumerate(chunks):
                        nc.tensor.matmul(out=ps[:], lhsT=ch, rhs=rhs_tile[:, i, :nw],
                                         start=(i == 0), stop=(i == n - 1))

                def bias_relu_T(ps, bias_ap, w, relu, nT, tagb):
                    zs = p4.tile([GPC, w], F16, tag="z" + tagb)
                    if relu:
                        nc.vector.tensor_tensor(out=zs[:], in0=ps[:],
                                                in1=bias_ap, op=ALU.add)
                        nc.vector.tensor_scalar(out=zs[:], in0=zs[:], scalar1=0.0,
                                                scalar2=None, op0=ALU.max)
                    zT = p4.tile([128, nT, GPC], F16, tag="zT" + tagb)
                    for i in range(nT):
                        psT2 = p4t.tile([128, 128], F16, space="PSUM", tag="psT4h")
                        nc.tensor.transpose(out=psT2[:, :GPC],
                                            in_=zs[:, 128 * i:128 * (i + 1)],
                                            identity=ident_s[:GPC, :GPC])
                        nc.scalar.activation(out=zT[:, i, :], in_=psT2[:, :GPC],
                                             func=AF.Copy)
                    return zT

                ps_z12 = p4s.tile([GPC, 256], F32, space="PSUM", tag="psz12")
                head_mm(ps_z12, [gmaxT[:, j, :] for j in range(7)]
                        + [gmeanT[:, j, :] for j in range(7)], wfg_s, 256)
                z12T = bias_relu_T(ps_z12, bias_s['bfg12'][:], 256, True, 2, "12")
                ps_h1 = p4s.tile([GPC, 512], F32, space="PSUM", tag="psh1")
                head_mm(ps_h1, [z12T[:, 0, :], z12T[:, 1, :], xtT_s[:, 0, :],
                                xtT_s[:, 1, :]], w1_s, 512)
                h1T = bias_relu_T(ps_h1, bias_s['b1'][:], 512, True, 4, "h1")
                ps_h2 = p4s.tile([GPC, 256], F32, space="PSUM", tag="psh2")
                head_mm(ps_h2, [h1T[:, i, :] for i in range(4)], w2_s, 256)
                h2T = bias_relu_T(ps_h2, bias_s['b2'][:], 256, True, 2, "h2")
                ps_o = p4s.tile([GPC, 1], F32, space="PSUM", tag="pso")
                head_mm(ps_o, [h2T[:, i, :] for i in range(2)], wo_s, 1)
                o_s = p4.tile([GPC, 1], F32, tag="os")
                nc.vector.tensor_scalar(out=o_s[:], in0=ps_o[:], scalar1=bo_s[:],
                                        scalar2=None, op0=ALU.add)
                nc.sync.dma_start(out=out_d[:], in_=o_s[:])

    nc.compile()
    return nc


def build_in_maps(nc, shared, cores):
    declared = set()
    import concourse.mybir as _mb
    for alloc in nc.m.functions[0].allocations:
        if isinstance(alloc, _mb.MemoryLocationSet) and alloc.kind == "ExternalInput":
            declared.add(alloc.memorylocations[0].name)
    in_maps = []
    for c in range(8):
        m = dict(shared)
        m.update(cores[c])
        in_maps.append({k: np.ascontiguousarray(v) for k, v in m.items()
                        if k in declared})
    return in_maps


_CACHE = {}


def run_device(inputs):
    meta, shared, cores = prep(**inputs)
    key = (meta['NBLK'], meta['TPB'], meta['PW'])
    if key not in _CACHE:
        _CACHE[key] = build(meta)
    nc = _CACHE[key]
    in_maps = build_in_maps(nc, shared, cores)
    res = run_bass_kernel_spmd(nc, in_maps, core_ids=list(range(8)))
    out = np.concatenate([res.results[c]['out'] for c in range(8)], axis=0)
    return out.astype(np.float32)


def kernel(**inputs):
    return run_device(inputs)
